# revision 14
# baseline (speedup 1.0000x reference)
"""Trainium2 Bass kernel v2 for nn_AttentionBlock: fp8 DoubleRow matmuls +
restructured softmax/normalize.

Data-parallel over batch (2 items/core, 8 cores), no collectives.

v2 changes vs v1:
- All dense matmuls (qkv, cond kv, value, proj) run fp8e4m3 with
  perf_mode=DoubleRow (2 k-tiles per instruction, 2x PE throughput).
  Scores stay fp16 (DoubleRow can't help K=64-per-head contractions
  without a layout that costs the same in weight loads).
- Attention runs per-head with a 1-event software lag: score chunk-pairs
  land in [128,2,512] PSUM tiles, one exp per tile (FD=1024) writes the
  fp8e5m2 probability pair-tile that value DoubleRow matmuls consume.
- Softmax denominators (ones-column of the value stationary) are
  reciprocal'd in place on partition 64, bounced via a DRAM scratch row,
  and broadcast-read back to 64 partitions by the (FIFO) SWDGE queue --
  no PE broadcast matmuls, no per-parity PSUM round trips.
- GroupNorm rstd = exp(-0.5*ln(var+eps)): keeps the whole kernel in the
  natural_log_exp activation table set (no table-switch stalls).
"""
import numpy as np
import concourse.bass as bass
import concourse.mybir as mybir
import concourse.tile as tile
from concourse.bass_utils import run_bass_kernel_spmd

B, C, HS, WS = 16, 512, 32, 32
T = HS * WS          # 1024 spatial positions
HEADS, HC = 8, 64
L = 77               # cond sequence length
CD = 768             # cond dim
S = T + L            # 1101 key positions
GROUPS = 32
GSIZE = C // GROUPS
EPS = 1e-5
N_CORES = 8
BPC = B // N_CORES   # batch items per core
CT = C // 128        # 4 channel tiles
CCT = CD // 128      # 6 cond channel tiles
NSC = 9              # key chunks: 8 x 128 + 77
NSP = 4              # full chunk pairs
TH = 2               # t halves of 512
SLOT = 66            # vT head slot width (64 vals + ones + pad), 8*66=528

F8 = mybir.dt.float8e4
F8E5 = mybir.dt.float8e5
F16 = mybir.dt.float16
F32 = mybir.dt.float32
AF = mybir.ActivationFunctionType
AL = mybir.AluOpType
DR = mybir.MatmulPerfMode.DoubleRow
LN2_4 = float(4.0 * np.log(2.0))

_CACHE = {}


def split_multi_waits(nc):
    """walrus in this container accepts at most one sync wait per
    instruction; hoist extra waits onto preceding NOPs on the same engine."""
    n_split = 0
    for f in nc.m.functions:
        for blk in f.blocks:
            new_insts = []
            for inst in blk.instructions:
                si = inst.sync_info
                if si is not None and si.on_wait is not None and len(si.on_wait) > 1:
                    waits = list(si.on_wait)
                    for w in waits[:-1]:
                        nop = mybir.InstNoOp(
                            name=f"{inst.name}-wsplit{n_split}",
                            ins=[], outs=[],
                            sync_info=mybir.SyncInfo(on_wait=[w], on_update=[]),
                        )
                        nop.engine = inst.engine
                        new_insts.append(nop)
                        n_split += 1
                    si.on_wait = [waits[-1]]
                    inst.sync_info = si
                new_insts.append(inst)
            blk.instructions = new_insts
    return n_split


def dedupe_ldweights(nc):
    """Delete an InstLdweights when the previous one loaded the identical
    stationary and only InstMatmult instructions sit between (so neither the
    PE stationary register nor the SBUF weights bytes can have changed).
    The deleted LDW's sems move onto the following instruction."""
    n_del = 0
    for f in nc.m.functions:
        for blk in f.blocks:
            new_insts = []
            last_key = None
            only_mms = False
            pending_sync = None
            for inst in blk.instructions:
                if pending_sync is not None:
                    si = inst.sync_info or mybir.SyncInfo(on_wait=[], on_update=[])
                    si.on_wait = list(si.on_wait or []) + list(pending_sync.on_wait or [])
                    si.on_update = list(si.on_update or []) + list(pending_sync.on_update or [])
                    inst.sync_info = si
                    pending_sync = None
                if isinstance(inst, mybir.InstLdweights):
                    key = (str(inst.ins[0]), str(getattr(inst, "perf_mode", None)),
                           str(getattr(inst, "tile_position", None)),
                           str(getattr(inst, "tile_size", None)))
                    if key == last_key and only_mms:
                        if inst.sync_info is not None and (
                                inst.sync_info.on_wait or inst.sync_info.on_update):
                            pending_sync = inst.sync_info
                        n_del += 1
                        continue
                    last_key = key
                    only_mms = True
                elif isinstance(inst, (mybir.InstMatmult, mybir.InstNoOp)):
                    pass
                else:
                    only_mms = False
                new_insts.append(inst)
            blk.instructions = new_insts
    return n_del


def build_program(apply_vbias=False, apply_pbias=False, repeat=1):
    nc = bass.Bass("TRN2", target_bir_lowering=False, debug=False, num_devices=1)

    xd = nc.dram_tensor("x_sh", [BPC, C, T], F32, kind="ExternalInput")
    cd = nc.dram_tensor("c_sh", [BPC, CD, L], F8, kind="ExternalInput")
    wqd = nc.dram_tensor("wqT", [C, C], F8, kind="ExternalInput")
    wkd = nc.dram_tensor("wkT", [C, C], F8, kind="ExternalInput")
    wvd = nc.dram_tensor("wvT", [C, C], F8, kind="ExternalInput")
    wkcd = nc.dram_tensor("wkcT", [CD, C], F8, kind="ExternalInput")
    wvcd = nc.dram_tensor("wvcT", [CD, C], F8, kind="ExternalInput")
    wpd = nc.dram_tensor("wpT", [C, C], F8, kind="ExternalInput")
    Gd = nc.dram_tensor("G", [128, CT, GROUPS], F32, kind="ExternalInput")
    GTd = nc.dram_tensor("GT", [GROUPS, CT, 128], F32, kind="ExternalInput")
    qbd = nc.dram_tensor("qb", [128, CT], F32, kind="ExternalInput")
    kbd = nc.dram_tensor("kb", [128, CT], F32, kind="ExternalInput")
    kcbd = nc.dram_tensor("kcb", [128, CT], F32, kind="ExternalInput")
    pbd = nc.dram_tensor("pb", [128, CT], F32, kind="ExternalInput")
    vbd = nc.dram_tensor("vbrow", [1, HEADS * 64], F16, kind="ExternalInput")
    rscr = nc.dram_tensor("r_scr", [BPC, HEADS, T], F16, kind="Internal")
    outd = nc.dram_tensor("out", [BPC, C, T], F32, kind="ExternalOutput")

    with tile.TileContext(nc) as tc:
        with tc.tile_pool(name="wp", bufs=1) as wp, \
             tc.tile_pool(name="xp", bufs=4) as xp, \
             tc.tile_pool(name="gnp", bufs=8) as gnp, \
             tc.tile_pool(name="qkp", bufs=8) as qkp, \
             tc.tile_pool(name="vtp", bufs=10) as vtp, \
             tc.tile_pool(name="ptp", bufs=6) as ptp, \
             tc.tile_pool(name="app", bufs=4) as app, \
             tc.tile_pool(name="nmp", bufs=3) as nmp, \
             tc.tile_pool(name="scp", bufs=2, space="PSUM") as scp, \
             tc.tile_pool(name="acp", bufs=2, space="PSUM") as acp:

            x_sb = {}
            c_sb = {}

            def emit_input_dmas(rep):
                for b in range(BPC):
                    xt = xp.tile([128, CT, T], F32, name=f"x_{rep}_{b}", tag="x")
                    nc.sync.dma_start(
                        xt[:], xd.ap()[b].rearrange("(a p) t -> p a t", p=128))
                    for m in range(CT):
                        x_sb[(b, m)] = xt
                    t_ = gnp.tile([128, CCT, 80], F8, name=f"c_{rep}_{b}", tag="c")
                    nc.sync.dma_start(t_[:, :, 0:L],
                                      cd.ap()[b].rearrange("(a p) l -> p a l", p=128))
                    c_sb[b] = t_

            emit_input_dmas(0)

            # ---- weights & constants -------------------------------------
            wq_sb = wp.tile([128, CT, C], F8, name="wq_sb")
            wk_sb = wp.tile([128, CT, C], F8, name="wk_sb")
            wv_sb = wp.tile([128, CT, C], F8, name="wv_sb")
            wkc_sb = wp.tile([128, CCT, C], F8, name="wkc_sb")
            wvc_sb = wp.tile([128, CCT, C], F8, name="wvc_sb")
            wp_sb = wp.tile([128, CT, C], F8, name="wp_sb")
            nc.sync.dma_start(wq_sb[:], wqd.ap().rearrange("(a p) o -> p a o", p=128))
            nc.sync.dma_start(wk_sb[:], wkd.ap().rearrange("(a p) o -> p a o", p=128))
            nc.sync.dma_start(wv_sb[:], wvd.ap().rearrange("(a p) o -> p a o", p=128))
            nc.sync.dma_start(wkc_sb[:], wkcd.ap().rearrange("(a p) o -> p a o", p=128))
            nc.sync.dma_start(wvc_sb[:], wvcd.ap().rearrange("(a p) o -> p a o", p=128))
            nc.sync.dma_start(wp_sb[:], wpd.ap().rearrange("(a p) o -> p a o", p=128))
            G_sb = wp.tile([128, CT, GROUPS], F32, name="G_sb")
            GT_sb = wp.tile([GROUPS, CT, 128], F32, name="GT_sb")
            nc.sync.dma_start(G_sb[:], Gd.ap())
            nc.sync.dma_start(GT_sb[:], GTd.ap())
            qb_sb = wp.tile([128, CT], F32, name="qb_sb")
            kb_sb = wp.tile([128, CT], F32, name="kb_sb")
            kcb_sb = wp.tile([128, CT], F32, name="kcb_sb")
            pb_sb = wp.tile([128, CT], F32, name="pb_sb")
            nc.sync.dma_start(qb_sb[:], qbd.ap())
            nc.sync.dma_start(kb_sb[:], kbd.ap())
            nc.sync.dma_start(kcb_sb[:], kcbd.ap())
            nc.sync.dma_start(pb_sb[:], pbd.ap())
            eps_sb = wp.tile([GROUPS, 1], F32, name="eps_sb")
            nc.vector.memset(eps_sb[:], EPS)
            neghalf_sb = wp.tile([GROUPS, 1], F32, name="neghalf_sb")
            nc.vector.memset(neghalf_sb[:], -0.5)
            zero_sb = wp.tile([GROUPS, 1], F32, name="zero_sb")
            nc.vector.memset(zero_sb[:], 0.0)
            ebias_sb = wp.tile([128, 1], F32, name="ebias_sb")
            nc.vector.memset(ebias_sb[:], -LN2_4)
            if apply_vbias:
                vb_bc = wp.tile([128, HEADS, 64], F16, name="vb_bc")
                vap = vbd.ap()
                nc.gpsimd.dma_start(
                    out=vb_bc[:],
                    in_=bass.AP(tensor=vap.tensor, offset=0,
                                ap=[[0, 128], [1, HEADS * 64]]),
                )

            for _rep in range(repeat):
                if _rep > 0:
                    x_sb = {}
                    c_sb = {}
                    emit_input_dmas(_rep)

                # ---- GroupNorm stats for both batches --------------------
                E_sb = {}
                for b in range(BPC):
                    mv_all = gnp.tile([128, CT, 2], F32,
                                      name=f"mv_{_rep}_{b}", tag="mv")
                    s12_all = gnp.tile([128, CT, 2], F32,
                                       name=f"s12_{_rep}_{b}", tag="s12")
                    for m in range(CT):
                        st = gnp.tile([128, 2, 6], F32,
                                      name=f"bnst_{_rep}_{b}_{m}", tag="bnst")
                        for sg in range(2):
                            nc.vector.bn_stats(
                                out=st[:, sg, :],
                                in_=x_sb[(b, m)][:, m, 512 * sg:512 * (sg + 1)])
                        nc.vector.bn_aggr(out=mv_all[:, m, :], in_=st[:])
                    nc.vector.tensor_copy(s12_all[:, :, 0:1], mv_all[:, :, 0:1])
                    nc.vector.tensor_tensor(
                        out=s12_all[:, :, 1:2], in0=mv_all[:, :, 0:1],
                        in1=mv_all[:, :, 0:1], op=AL.mult)
                    nc.vector.tensor_tensor(
                        out=s12_all[:, :, 1:2], in0=s12_all[:, :, 1:2],
                        in1=mv_all[:, :, 1:2], op=AL.add)
                    st_ps = acp.tile([GROUPS, 2], F32, name=f"stps_{_rep}_{b}",
                                     tag="acc", padded_shape=[128, T])
                    for m in range(CT):
                        nc.tensor.matmul(st_ps[:], G_sb[:, m, :],
                                         s12_all[:, m, :],
                                         start=(m == 0), stop=(m == CT - 1))
                    grp = gnp.tile([GROUPS, 4], F32, name=f"grp_{_rep}_{b}",
                                   tag="grp")
                    # cols: 1=rstd 2=mean 3=var ([1:3]=[rstd,mean] feeds E)
                    nc.vector.tensor_scalar_mul(grp[:, 2:4], in0=st_ps[:],
                                                scalar1=1.0 / GSIZE)
                    nc.vector.tensor_tensor(out=grp[:, 1:2], in0=grp[:, 2:3],
                                            in1=grp[:, 2:3], op=AL.mult)
                    nc.vector.tensor_tensor(out=grp[:, 3:4], in0=grp[:, 3:4],
                                            in1=grp[:, 1:2], op=AL.subtract)
                    # rstd = exp(-0.5 * ln(var + eps)) -- stays in the
                    # natural_log_exp table set alongside the softmax exp
                    nc.scalar.activation(grp[:, 3:4], grp[:, 3:4], AF.Ln,
                                         bias=eps_sb[:])
                    nc.scalar.activation(grp[:, 1:2], grp[:, 3:4], AF.Exp,
                                         bias=zero_sb[:], scale=neghalf_sb[:])
                    e_ps = acp.tile([128, CT, 2], F32,
                                    name=f"eps_{_rep}_{b}", tag="acc",
                                    padded_shape=[128, CT, 256])
                    for m in range(CT):
                        nc.tensor.matmul(e_ps[:, m, :], GT_sb[:, m, :],
                                         grp[:, 1:3], start=True, stop=True)
                    e_t = gnp.tile([128, CT, 2], F32,
                                   name=f"E_{_rep}_{b}", tag="E")
                    nc.vector.tensor_copy(e_t[:], e_ps[:])
                    for m in range(CT):
                        E_sb[(b, m)] = e_t

                # ---- per-batch pipeline ----------------------------------
                for b in range(BPC):
                    # xn = (x - mean) * rstd, fp8, one tile [128, CT, T]
                    xn = qkp.tile([128, CT, T], F8, name=f"xn_{_rep}_{b}",
                                  tag="xn")
                    for m in range(CT):
                        nc.vector.tensor_scalar(
                            out=xn[:, m, :], in0=x_sb[(b, m)][:, m, :],
                            scalar1=E_sb[(b, m)][:, m, 1:2],
                            scalar2=E_sb[(b, m)][:, m, 0:1],
                            op0=AL.subtract, op1=AL.mult)

                    q_sb = {}
                    k_sb = {}

                    def emit_qk(m):
                        q_ps = scp.tile([128, 2, 512], F32,
                                        name=f"qps_{_rep}_{b}_{m}", tag="sc")
                        qv = q_ps.rearrange("p a b -> p (a b)")
                        for i in range(2):
                            for th in range(TH):
                                tsl = slice(512 * th, 512 * (th + 1))
                                nc.tensor.matmul(
                                    qv[:, tsl],
                                    wq_sb[:, 2 * i:2 * i + 2,
                                          128 * m:128 * (m + 1)],
                                    xn[:, 2 * i:2 * i + 2, tsl],
                                    start=(i == 0), stop=(i == 1),
                                    perf_mode=DR)
                        qt = qkp.tile([128, T], F16,
                                      name=f"q_{_rep}_{b}_{m}", tag="q")
                        nc.vector.tensor_scalar(out=qt[:], in0=qv[:],
                                                scalar1=qb_sb[:, m:m + 1],
                                                scalar2=None, op0=AL.add)
                        q_sb[m] = qt
                        k_ps = scp.tile([128, 2, 512], F32,
                                        name=f"kps_{_rep}_{b}_{m}", tag="sc")
                        kv = k_ps.rearrange("p a b -> p (a b)")
                        for i in range(2):
                            for th in range(TH):
                                tsl = slice(512 * th, 512 * (th + 1))
                                nc.tensor.matmul(
                                    kv[:, tsl],
                                    wk_sb[:, 2 * i:2 * i + 2,
                                          128 * m:128 * (m + 1)],
                                    xn[:, 2 * i:2 * i + 2, tsl],
                                    start=(i == 0), stop=(i == 1),
                                    perf_mode=DR)
                        kt = qkp.tile([128, S], F16,
                                      name=f"k_{_rep}_{b}_{m}", tag="k")
                        nc.vector.tensor_scalar(out=kt[:, 0:T], in0=kv[:],
                                                scalar1=kb_sb[:, m:m + 1],
                                                scalar2=None, op0=AL.add)
                        k_sb[m] = kt
                        kc_ps = scp.tile([128, 2, 512], F32,
                                         name=f"kcps_{_rep}_{b}_{m}", tag="sc")
                        kcv = kc_ps.rearrange("p a b -> p (a b)")
                        for i in range(CCT // 2):
                            nc.tensor.matmul(
                                kcv[:, 0:L],
                                wkc_sb[:, 2 * i:2 * i + 2,
                                       128 * m:128 * (m + 1)],
                                c_sb[b][:, 2 * i:2 * i + 2, 0:L],
                                start=(i == 0), stop=(i == CCT // 2 - 1),
                                perf_mode=DR)
                        nc.vector.tensor_scalar(out=k_sb[m][:, T:S],
                                                in0=kcv[:, 0:L],
                                                scalar1=kcb_sb[:, m:m + 1],
                                                scalar2=None, op0=AL.add)

                    for m in range(CT):
                        emit_qk(m)

                    # vT: [key_pos, pair, 8 heads x 66-slot (64 vals + ones)]
                    vT = {}
                    for s in range(NSC):
                        sp, half = s // 2, s % 2
                        if half == 0:
                            vt = vtp.tile([128, 2, HEADS * SLOT], F8,
                                          name=f"vt_{_rep}_{b}_{sp}", tag="vt")
                            vT[sp] = vt
                        vt = vT[sp]
                        pv = acp.tile([128 if s < 8 else L, 512], F32,
                                      name=f"pv_{_rep}_{b}_{s}", tag="acc",
                                      padded_shape=[128, T])
                        if s < 8:
                            for i in range(2):
                                nc.tensor.matmul(
                                    pv[:],
                                    xn[:, 2 * i:2 * i + 2,
                                       128 * s:128 * (s + 1)],
                                    wv_sb[:, 2 * i:2 * i + 2, :],
                                    start=(i == 0), stop=(i == 1),
                                    perf_mode=DR)
                        else:
                            for i in range(CCT // 2):
                                nc.tensor.matmul(
                                    pv[:],
                                    c_sb[b][:, 2 * i:2 * i + 2, 0:L],
                                    wvc_sb[:, 2 * i:2 * i + 2, :],
                                    start=(i == 0), stop=(i == CCT // 2 - 1),
                                    perf_mode=DR)
                        sdim = 128 if s < 8 else L
                        vt_view = vt.rearrange("p two (h c) -> p two h c",
                                               c=SLOT)
                        pv_view = pv.rearrange("p (h c) -> p h c", c=64)
                        nc.vector.tensor_copy(vt_view[0:sdim, half, :, 0:64],
                                              pv_view[:])
                        if apply_vbias:
                            nc.vector.tensor_tensor(
                                out=vt_view[0:sdim, half, :, 0:64],
                                in0=vt_view[0:sdim, half, :, 0:64],
                                in1=vb_bc[0:sdim],
                                op=AL.add)
                        if half == 1 or s == 8:
                            nc.vector.memset(vt_view[:, :, :, 64:65], 1.0)

                    apair = {}
                    for pi in range(2):
                        apair[pi] = app.tile([128, 2, T], F8,
                                             name=f"ap_{_rep}_{b}_{pi}",
                                             tag="apair")

                    # ---- attention: per-head, 1-event lag ----------------
                    LAG = 2
                    events = [(h, sp) for h in range(HEADS)
                              for sp in range(NSP + 1)]
                    state = {}

                    def head_rows(h):
                        off = 64 * (h % 2)
                        return h // 2, off, slice(off, off + 64)

                    def emit_scores(h, sp):
                        m, off, rows = head_rows(h)
                        if sp < NSP:
                            pt = ptp.tile([128, 2, T], F8E5,
                                          name=f"pt_{_rep}_{b}_{h}_{sp}",
                                          tag="pt")
                            scs = [scp.tile([128, 2, 512], F32,
                                            name=f"sc_{_rep}_{b}_{h}_{sp}_{th}",
                                            tag="sc") for th in range(TH)]
                            for ci in range(2):
                                s0 = 128 * (2 * sp + ci)
                                for th in range(TH):
                                    tsl = slice(512 * th, 512 * (th + 1))
                                    nc.tensor.matmul(
                                        scs[th][:, ci, :],
                                        k_sb[m][rows, s0:s0 + 128],
                                        q_sb[m][rows, tsl],
                                        start=True, stop=True,
                                        tile_position=(off, 0))
                            for th in range(TH):
                                tsl = slice(512 * th, 512 * (th + 1))
                                nc.scalar.activation(pt[:, :, tsl], scs[th][:],
                                                     AF.Exp, bias=ebias_sb[:])
                            state[(h, sp)] = pt
                        else:
                            sc = scp.tile([128, 2, 512], F32,
                                          name=f"scc_{_rep}_{b}_{h}", tag="sc")
                            scv = sc.rearrange("p a b -> p (a b)")
                            for th in range(TH):
                                tsl = slice(512 * th, 512 * (th + 1))
                                nc.tensor.matmul(scv[0:L, tsl],
                                                 k_sb[m][rows, T:S],
                                                 q_sb[m][rows, tsl],
                                                 start=True, stop=True,
                                                 tile_position=(off, 0))
                            ptc = ptp.tile([128, T], F8E5,
                                           name=f"ptc_{_rep}_{b}_{h}",
                                           tag="ptc")
                            nc.scalar.activation(ptc[0:L, :], scv[0:L, :],
                                                 AF.Exp, bias=ebias_sb[0:L, :])
                            state[(h, sp)] = ptc

                    def emit_value(h, sp):
                        if sp == 0:
                            a_ps = acp.tile([128, T], F32,
                                            name=f"a_{_rep}_{b}_{h}",
                                            tag="acc")
                            state[("a", h)] = a_ps
                        a_ps = state[("a", h)]
                        pt = state.pop((h, sp))
                        slot = slice(SLOT * h, SLOT * h + 65)
                        for th in range(TH):
                            tsl = slice(512 * th, 512 * (th + 1))
                            if sp < NSP:
                                nc.tensor.matmul(a_ps[0:65, tsl],
                                                 vT[sp][:, :, slot],
                                                 pt[:, :, tsl],
                                                 start=(sp == 0), stop=False,
                                                 perf_mode=DR)
                            else:
                                nc.tensor.matmul(a_ps[0:65, tsl],
                                                 vT[4][0:L, 0, slot],
                                                 pt[0:L, tsl],
                                                 start=False, stop=True)

                    def emit_normalize(h):
                        a_ps = state.pop(("a", h))
                        rr = nmp.tile([65, T], F16, name=f"rr_{_rep}_{b}_{h}",
                                      tag="rr", padded_shape=[128, T])
                        with nc.allow_low_precision("denom recip fp16"):
                            nc.vector.reciprocal(rr[64:65, :], a_ps[64:65, :])
                        nc.gpsimd.dma_start(out=rscr.ap()[b, h:h + 1, :],
                                            in_=rr[64:65, :])
                        rbc = nmp.tile([64, T], F16, name=f"rb_{_rep}_{b}_{h}",
                                       tag="rbc")
                        srow = rscr.ap()[b, h:h + 1, :]
                        nc.gpsimd.dma_start(
                            out=rbc[:],
                            in_=bass.AP(tensor=srow.tensor, offset=srow.offset,
                                        ap=[[0, 64], [1, T]]),
                        )
                        pi, half = h // 4, (h // 2) % 2
                        if h % 2 == 0:
                            nc.vector.tensor_tensor(
                                out=apair[pi][0:64, half, :],
                                in0=a_ps[0:64, :], in1=rbc[:], op=AL.mult)
                        else:
                            ao = nmp.tile([64, T], F8,
                                          name=f"ao_{_rep}_{b}_{h}", tag="ao")
                            nc.vector.tensor_tensor(
                                out=ao[:], in0=a_ps[0:64, :], in1=rbc[:],
                                op=AL.mult)
                            nc.sync.dma_start(apair[pi][64:128, half, :],
                                              ao[:])

                    for idx in range(len(events) + LAG):
                        if idx < len(events):
                            emit_scores(*events[idx])
                        if idx >= LAG:
                            h, sp = events[idx - LAG]
                            emit_value(h, sp)
                            if sp == NSP:
                                emit_normalize(h)

                    # ---- projection + residual ---------------------------
                    for m in range(CT):
                        h_ps = acp.tile([128, T], F32,
                                        name=f"hps_{_rep}_{b}_{m}", tag="acc")
                        for pi in range(2):
                            for th in range(TH):
                                tsl = slice(512 * th, 512 * (th + 1))
                                nc.tensor.matmul(
                                    h_ps[:, tsl],
                                    wp_sb[:, 2 * pi:2 * pi + 2,
                                          128 * m:128 * (m + 1)],
                                    apair[pi][:, :, tsl],
                                    start=(pi == 0), stop=(pi == 1),
                                    perf_mode=DR)
                        nc.vector.tensor_tensor(out=x_sb[(b, m)][:, m, :],
                                                in0=h_ps[:],
                                                in1=x_sb[(b, m)][:, m, :],
                                                op=AL.add)
                        if apply_pbias:
                            nc.vector.tensor_scalar(out=x_sb[(b, m)][:, m, :],
                                                    in0=x_sb[(b, m)][:, m, :],
                                                    scalar1=pb_sb[:, m:m + 1],
                                                    scalar2=None, op0=AL.add)
                        nc.sync.dma_start(outd.ap()[b, 128 * m:128 * (m + 1), :],
                                          x_sb[(b, m)][:, m, :])

    dedupe_ldweights(nc)
    split_multi_waits(nc)
    return nc


def _prepare(inputs):
    x = np.asarray(inputs["x"], np.float32).reshape(B, C, T)
    c = np.asarray(inputs["c"], np.float32)
    gamma = np.asarray(inputs["gamma"], np.float32)
    beta = np.asarray(inputs["beta"], np.float32)
    w_qkv = np.asarray(inputs["w_qkv"], np.float32)
    b_qkv = np.asarray(inputs["b_qkv"], np.float32)
    w_c = np.asarray(inputs["w_c"], np.float32)
    b_c = np.asarray(inputs["b_c"], np.float32)
    w_p = np.asarray(inputs["w_p"], np.float32)
    b_p = np.asarray(inputs["b_p"], np.float32)

    scale = 1.0 / np.sqrt(HC)  # 0.125, exact
    wq = w_qkv[0:C] * gamma[None, :]
    wk = w_qkv[C:2 * C] * gamma[None, :] * scale
    wv = w_qkv[2 * C:3 * C] * gamma[None, :]
    qb = w_qkv[0:C] @ beta + b_qkv[0:C]
    kb = (w_qkv[C:2 * C] @ beta + b_qkv[C:2 * C]) * scale
    vb = w_qkv[2 * C:3 * C] @ beta + b_qkv[2 * C:3 * C]
    wkc = w_c[0:C] * scale
    kcb = b_c[0:C] * scale
    wvc = w_c[C:2 * C]
    vcb = b_c[C:2 * C]

    def colsplit(v):  # [512] -> [128, 4] per-channel-tile columns
        return np.ascontiguousarray(v.reshape(CT, 128).T).astype(np.float32)

    G = np.zeros((128, CT, GROUPS), np.float32)
    GT = np.zeros((GROUPS, CT, 128), np.float32)
    for m in range(CT):
        for p in range(128):
            g = (m * 128 + p) // GSIZE
            G[p, m, g] = 1.0
            GT[g, m, p] = 1.0

    apply_vbias = bool(np.any(vb != 0) or np.any(vcb != 0))
    if apply_vbias and not np.allclose(vb, vcb):
        raise NotImplementedError("distinct self/cond v biases not supported")
    apply_pbias = bool(np.any(b_p != 0))

    f8 = mybir.dt.np(F8)

    def to_f8(w):
        return np.clip(np.ascontiguousarray(w.T), -240, 240).astype(f8)

    shared = {
        "wqT": to_f8(wq),
        "wkT": to_f8(wk),
        "wvT": to_f8(wv),
        "wkcT": to_f8(wkc),
        "wvcT": to_f8(wvc),
        "wpT": to_f8(w_p),
        "G": G, "GT": GT,
        "qb": colsplit(qb), "kb": colsplit(kb), "kcb": colsplit(kcb),
        "pb": colsplit(b_p),
        "vbrow": vb.reshape(1, HEADS * 64).astype(np.float16),
    }
    in_maps = []
    c8 = np.clip(c, -240, 240).astype(f8)
    for core in range(N_CORES):
        m = dict(shared)
        m["x_sh"] = np.ascontiguousarray(x[BPC * core:BPC * (core + 1)])
        m["c_sh"] = np.ascontiguousarray(c8[BPC * core:BPC * (core + 1)])
        in_maps.append(m)
    return in_maps, apply_vbias, apply_pbias


def run(inputs, trace=False):
    in_maps, avb, apb = _prepare(inputs)
    key = (avb, apb)
    if key not in _CACHE:
        _CACHE[key] = build_program(apply_vbias=avb, apply_pbias=apb)
    nc = _CACHE[key]
    res = run_bass_kernel_spmd(nc, in_maps, core_ids=list(range(N_CORES)),
                               trace=trace)
    out = np.concatenate([res.results[c]["out"] for c in range(N_CORES)], axis=0)
    return out.reshape(B, C, HS, WS).astype(np.float32), res


def kernel(**inputs):
    out, _ = run(inputs, trace=False)
    return out
